# revision 6
# baseline (speedup 1.0000x reference)
"""Trainium2 Bass kernel for nn_CriterionPairWiseforWholeFeatAfterPool.

Computation (reference): select feat_ind slice -> MaxPool2d with kernel
(H/2, W/2) producing a 2x2 pooled map per (sample, channel) -> L2-normalize
over channels -> per-sample 4x4 gram over the pooled spatial positions ->
scalar MSE-style loss between teacher/student grams.

Strategy (data-parallel, per the sharding hint): shard the batch axis B=16
across 8 NeuronCores (2 samples/core, 64 MiB of HBM->SBUF traffic each).
Channels ride the 128 SBUF partitions; every 64x64 max-pool window reduces
on the vector engine (free-axis reduce_max over strided quadrant views);
partial-max columns fold on host in the tiny gram/loss epilogue.

Engine shaping: on even-numbered physical NeuronCores one edge SDMA engine
(idx 0 or idx 15) runs ~20% slower (pair-shared AXI port contention).
HWDGE assigns descriptors positionally (engine = position % 16, reset per
op), so engine 15's share is controllable by descriptor count: every
full-band op carries 127 descriptors (partitions 0..126) -> engine 15
gets 7 descriptors vs 8 everywhere else (12.5% shave, matching most of
the deficit).  The missing partition-127 band data streams separately as
8-row pieces relocated onto partitions 0..63 in two 64-descriptor flat
ops (engine-neutral).  Engine 0 cannot be shaved positionally; cores with
a slow engine 0 are bounded by it regardless of layout.

The final band streams as 4 slim 16-row chunks so the vector engine
finishes ~2.5 us after the last byte lands instead of a full band-reduce
late.
"""

import contextlib

import numpy as np

import concourse.bacc as bacc
import concourse.mybir as mybir
from concourse.bass_utils import run_bass_kernel_spmd

N_CORES = 8
P = 128           # SBUF partitions
B_LOC = 2         # samples per core (16 / 8)
C = 256           # channels
H = 128
W = 128
BAND = 64         # pooling-window rows per streamed tile
FREE = BAND * W   # f32 elements per partition per full tile (32 KiB)
TAIL_SPLIT = 4    # the last band streams as 4 slim 16-row chunks
NBUF = 5          # SBUF slots (5 x 32 KiB/partition = 160 KiB)

N_BANDS = B_LOC * 2 * (C // P) * (H // BAND)   # 16 full bands
N_XFERS = N_BANDS - 1 + TAIL_SPLIT             # 15 big + 4 slim band ops
N_RELOC = 2                                    # channel-127 relocation ops
N_COLS = N_XFERS * 2 + N_RELOC * 2             # pooled partial column pairs
COL_RELOC = N_XFERS * 2

_NC = None


def _xfer_meta():
    """Band-op metadata: (band_index, sub_row_offset, rows)."""
    metas = []
    for bi in range(N_BANDS):
        if bi < N_BANDS - 1:
            metas.append((bi, 0, BAND))
        else:
            rs = BAND // TAIL_SPLIT
            for k in range(TAIL_SPLIT):
                metas.append((bi, k * rs, rs))
    return metas


def _band_addr(bi):
    """band index -> (x_idx, b, cb, band) in stream order."""
    band = bi % (H // BAND)
    r = bi // (H // BAND)
    cb = r % (C // P)
    r //= C // P
    x = r % 2
    b = r // 2
    return x, b, cb, band


def _build_nc():
    """Build + compile the per-core SPMD Bass program (same NEFF on all cores)."""
    nc = bacc.Bacc("TRN2", target_bir_lowering=False, debug=False,
                   num_devices=N_CORES)
    s = nc.dram_tensor("s", [B_LOC, C, H, W], mybir.dt.float32,
                       kind="ExternalInput").ap()
    t = nc.dram_tensor("t", [B_LOC, C, H, W], mybir.dt.float32,
                       kind="ExternalInput").ap()
    out = nc.dram_tensor("pooled", [P, N_COLS], mybir.dt.float32,
                         kind="ExternalOutput").ap()

    # band ops: partitions 0..126 only (127 descriptors -> engine-15 shave)
    xfers = []
    for bi, r_off, rows in _xfer_meta():
        xi, b, cb, band = _band_addr(bi)
        x = (s, t)[xi]
        r0 = band * BAND + r_off
        src = x[b, cb * P:cb * P + 127, r0:r0 + rows, :]
        xfers.append((src.rearrange("c h w -> c (h w)"), rows * W, rows))
    n = len(xfers)
    assert n == N_XFERS

    # relocation ops: channel cb*128+127 of every band, as 8-row pieces on
    # partitions 0..63.  One op per tensor: dims (b, cb, band, k) -> 64
    # pieces of 8 rows; 64 descriptors -> flat 4/engine.
    relocs = []
    for x in (s, t):
        src = x[:, 127::128, :, :].rearrange(
            "b cb (band k h) w -> b cb band k (h w)", band=2, k=8)
        relocs.append(src)  # [2, 2, 2, 8, 1024]

    with contextlib.ExitStack() as ctx:
        bufs = [ctx.enter_context(
            nc.sbuf_tensor(f"buf{i}", [P, FREE], mybir.dt.float32))
            for i in range(NBUF)]
        rbufs = [ctx.enter_context(
            nc.sbuf_tensor(f"rbuf{i}", [64, 1024], mybir.dt.float32))
            for i in range(N_RELOC)]
        pooled = ctx.enter_context(
            nc.sbuf_tensor("pooled_sb", [P, N_COLS], mybir.dt.float32))
        dma_sems = [ctx.enter_context(nc.semaphore(f"dma_sem{i}"))
                    for i in range(NBUF)]
        rel_sems = [ctx.enter_context(nc.semaphore(f"rel_sem{i}"))
                    for i in range(N_RELOC)]
        out_sem = ctx.enter_context(nc.semaphore("out_sem"))
        red_sem = ctx.enter_context(nc.semaphore("red_sem"))
        block = ctx.enter_context(nc.Block())

        n_red = n + N_RELOC

        @block.sync
        def _(sync):
            # relocation ops first: small (256 KiB), cheap descriptor gen,
            # so bytes start flowing early while band-op descriptors build
            for i, src in enumerate(relocs):
                sync.dma_start(rbufs[i][:, :], src).then_inc(rel_sems[i], 16)
            for i, (src, free, _h) in enumerate(xfers):
                if i >= NBUF:
                    sync.wait_ge(red_sem, i - NBUF + 1)
                sync.dma_start(
                    bufs[i % NBUF][0:127, :free], src).then_inc(
                        dma_sems[i % NBUF], 16)
            sync.wait_ge(red_sem, n_red)
            sync.dma_start(out, pooled[:, :]).then_inc(out_sem, 16)
            sync.wait_ge(out_sem, 16)

        @block.vector
        def _(vector):
            # relocation reduces first (their data lands first)
            for i in range(N_RELOC):
                vector.wait_ge(rel_sems[i], 16)
                view = rbufs[i].rearrange("p (h j w) -> p j h w", j=2, w=64)
                vector.tensor_reduce(
                    pooled[0:64, COL_RELOC + 2 * i:COL_RELOC + 2 * i + 2],
                    view, axis=mybir.AxisListType.XY,
                    op=mybir.AluOpType.max).then_inc(red_sem, 1)
            for i, (_src, free, h) in enumerate(xfers):
                vector.wait_ge(dma_sems[i % NBUF], 16 * (i // NBUF + 1))
                view = bufs[i % NBUF][0:127, :free].rearrange(
                    "c (h j w) -> c j h w", h=h, j=2, w=64)
                vector.tensor_reduce(
                    pooled[0:127, 2 * i:2 * i + 2], view,
                    axis=mybir.AxisListType.XY,
                    op=mybir.AluOpType.max).then_inc(red_sem, 1)

    nc.compile()
    return nc


def get_nc():
    global _NC
    if _NC is None:
        _NC = _build_nc()
    return _NC


def make_in_maps(fS, fT):
    """Per-core input dicts: batch-sharded contiguous slices."""
    return [{"s": np.ascontiguousarray(fS[B_LOC * i:B_LOC * (i + 1)]),
             "t": np.ascontiguousarray(fT[B_LOC * i:B_LOC * (i + 1)])}
            for i in range(N_CORES)]


def finish(pooled_list):
    """Host epilogue: fold partial maxes, gram + normalize + loss."""
    B = B_LOC * N_CORES
    fS = np.full((B, C, 4), -np.inf)
    fT = np.full((B, C, 4), -np.inf)
    metas = _xfer_meta()
    for i, arr in enumerate(pooled_list):
        a = np.asarray(arr)  # [P, N_COLS]
        f = (fS, fT)
        for k, (bi, _r_off, _rows) in enumerate(metas):
            xi, bl, cb, band = _band_addr(bi)
            tgt = f[xi][i * B_LOC + bl, cb * P:cb * P + 127,
                        band * 2:band * 2 + 2]
            np.maximum(tgt, a[0:127, 2 * k:2 * k + 2], out=tgt)
        # relocation cols: tensor xi, piece q = ((b*2+cb)*2+band)*8+k of
        # channel cb*128+127, band rows
        for xi in range(2):
            rc = a[0:64, COL_RELOC + 2 * xi:COL_RELOC + 2 * xi + 2]
            for q in range(64):
                k = q % 8
                band = (q // 8) % 2
                cb = (q // 16) % 2
                bl = q // 32
                tgt = f[xi][i * B_LOC + bl, cb * P + 127,
                            band * 2:band * 2 + 2]
                np.maximum(tgt, rc[q], out=tgt)

    def sim(f):
        G = np.einsum('bcm,bcn->bmn', f, f)
        d = np.sqrt(np.einsum('bmm->bm', G)) + 1e-8
        return G / (d[:, :, None] * d[:, None, :])

    loss = ((sim(fT) - sim(fS)) ** 2).sum() / (4 * 4) / B
    return np.float32(loss)


def run_device(fS, fT, **spmd_kwargs):
    """Run the compiled program on the 8 cores; returns (pooled_list, results)."""
    res = run_bass_kernel_spmd(get_nc(), make_in_maps(fS, fT),
                               core_ids=list(range(N_CORES)), **spmd_kwargs)
    pooled_list = [res.results[i]["pooled"] for i in range(N_CORES)]
    return pooled_list, res


def kernel(preds_S, preds_T, feat_ind):
    fi = int(np.asarray(feat_ind))
    fS = np.ascontiguousarray(np.asarray(preds_S)[fi], dtype=np.float32)
    fT = np.ascontiguousarray(np.asarray(preds_T)[fi], dtype=np.float32)
    try:
        pooled_list, _ = run_device(fS, fT)
    except Exception:
        # one retry: a cold device occasionally reports a transient
        # NRT execution error on the very first NEFF launch
        pooled_list, _ = run_device(fS, fT)
    return finish(pooled_list)


# revision 10
# speedup vs baseline: 13.5059x; 13.5059x over previous
"""Trainium2 Bass kernel for nn_CriterionPairWiseforWholeFeatAfterPool.

Computation (reference): select feat_ind slice -> MaxPool2d with kernel
(H/2, W/2) producing a 2x2 pooled map per (sample, channel) -> L2-normalize
over channels -> per-sample 4x4 gram over the pooled spatial positions ->
scalar MSE-style loss between teacher/student grams.

Strategy (data-parallel, per the sharding hint): shard the batch axis B=16
across 8 NeuronCores (2 samples/core).  Each core streams its two feature
shards (2 samples x 256 ch x 128 x 128 f32 = 64 MiB) HBM->SBUF with
channels on partitions and reduces every 64x64 max-pool window on the
vector engine (free-axis reduce_max over a strided quadrant view).  Each
core emits its pooled (partial-max) features; the tiny epilogue (fold
partials, per-sample 4x4 gram, normalization, final sum == the all-reduce
of per-core partials) runs on host in a few microseconds of numpy.

Chunking: full 64-row pooling bands stream as single 4 MiB DMA ops
(32 KiB contiguous per partition -> 1 descriptor per partition, 8 per
SDMA engine; 128-descriptor ops split perfectly flat across the 16
engines).  The first band leads with a slim chunk so bytes flow before
the big-op descriptor generation finishes, and the last two bands taper
to 32/16-row chunks so the vector engine drains within ~3 us of the
last byte.  The block skips the gpsimd dge-drain at exit (gpsimd unused).
"""

import contextlib

import numpy as np

import concourse.bacc as bacc
import concourse.mybir as mybir
from concourse.bass_utils import run_bass_kernel_spmd

N_CORES = 8
P = 128           # SBUF partitions
B_LOC = 2         # samples per core (16 / 8)
C = 256           # channels
H = 128
W = 128
BAND = 64         # pooling-window rows per streamed tile (4 MiB tiles)
FREE = BAND * W   # f32 elements per partition per full tile (32 KiB)
NBUF = 5          # SBUF slots (5 x 32 KiB/partition = 160 KiB)

N_BANDS = B_LOC * 2 * (C // P) * (H // BAND)   # 16 full bands

# Row-chunking per band: the first band leads with a slim 16-row chunk so
# first bytes flow ~4 us earlier (128-descriptor 4 MiB ops spend ~5 us in
# descriptor generation before the first byte moves); the last two bands
# taper (32-row, then 16-row chunks) so the vector engine's reduce lag at
# stream end drains instead of adding a full 8.7 us band-reduce after the
# last byte lands.
_BAND_CHUNKS = {0: (16, 48), N_BANDS - 2: (32, 32),
                N_BANDS - 1: (32, 16, 16)}


def _xfer_meta():
    """Transfer list metadata: (band_index, sub_row_offset, rows)."""
    metas = []
    for bi in range(N_BANDS):
        off = 0
        for rows in _BAND_CHUNKS.get(bi, (BAND,)):
            metas.append((bi, off, rows))
            off += rows
    return metas


N_XFERS = len(_xfer_meta())
N_COLS = N_XFERS * 2                           # pooled cols (pairs)

_NC = None


def _band_addr(bi):
    """band index -> (x_idx, b, cb, band) in stream order."""
    band = bi % (H // BAND)
    r = bi // (H // BAND)
    cb = r % (C // P)
    r //= C // P
    x = r % 2
    b = r // 2
    return x, b, cb, band


def _build_nc():
    """Build + compile the per-core SPMD Bass program (same NEFF on all cores)."""
    nc = bacc.Bacc("TRN2", target_bir_lowering=False, debug=False,
                   num_devices=N_CORES)
    s = nc.dram_tensor("s", [B_LOC, C, H, W], mybir.dt.float32,
                       kind="ExternalInput").ap()
    t = nc.dram_tensor("t", [B_LOC, C, H, W], mybir.dt.float32,
                       kind="ExternalInput").ap()
    out = nc.dram_tensor("pooled", [P, N_COLS], mybir.dt.float32,
                         kind="ExternalOutput").ap()

    # transfer list: (2-D dram source AP, free elems, rows covered)
    xfers = []
    for bi, r_off, rows in _xfer_meta():
        xi, b, cb, band = _band_addr(bi)
        x = (s, t)[xi]
        r0 = band * BAND + r_off
        src = x[b, cb * P:(cb + 1) * P, r0:r0 + rows, :]
        xfers.append((src.rearrange("c h w -> c (h w)"), rows * W, rows))
    n = len(xfers)
    assert n == N_XFERS

    with contextlib.ExitStack() as ctx:
        bufs = [ctx.enter_context(
            nc.sbuf_tensor(f"buf{i}", [P, FREE], mybir.dt.float32))
            for i in range(NBUF)]
        pooled = ctx.enter_context(
            nc.sbuf_tensor("pooled_sb", [P, N_COLS], mybir.dt.float32))
        # one DMA-completion semaphore per buffer slot: at most one in-flight
        # DMA per semaphore (slot reuse is serialized through red_sem), so
        # concurrent DMAs never race on the same semaphore
        dma_sems = [ctx.enter_context(nc.semaphore(f"dma_sem{i}"))
                    for i in range(NBUF)]
        out_sem = ctx.enter_context(nc.semaphore("out_sem"))
        red_sem = ctx.enter_context(nc.semaphore("red_sem"))
        # gpsimd is unused; skip its expensive dge_drain at block exit
        block = ctx.enter_context(nc.Block(no_gpsimd_drain=True))

        @block.sync
        def _(sync):
            for i, (src, free, _h) in enumerate(xfers):
                if i >= NBUF:
                    # slot reuse: wait until the reduce of tile i-NBUF is done
                    sync.wait_ge(red_sem, i - NBUF + 1)
                sync.dma_start(
                    bufs[i % NBUF][:, :free], src).then_inc(
                        dma_sems[i % NBUF], 16)
            sync.wait_ge(red_sem, n)
            sync.dma_start(out, pooled[:, :]).then_inc(out_sem, 16)
            sync.wait_ge(out_sem, 16)

        @block.vector
        def _(vector):
            for i, (_src, free, h) in enumerate(xfers):
                vector.wait_ge(dma_sems[i % NBUF], 16 * (i // NBUF + 1))
                # free dim is (h, w) row-major; expose the two 64-wide halves
                # as an outer axis, reduce the h x 64 window per half
                view = bufs[i % NBUF][:, :free].rearrange(
                    "c (h j w) -> c j h w", h=h, j=2, w=64)
                vector.tensor_reduce(
                    pooled[:, 2 * i:2 * i + 2], view,
                    axis=mybir.AxisListType.XY,
                    op=mybir.AluOpType.max).then_inc(red_sem, 1)

    nc.compile()
    return nc


def get_nc():
    global _NC
    if _NC is None:
        _NC = _build_nc()
    return _NC


def make_in_maps(fS, fT):
    """Per-core input dicts: batch-sharded contiguous slices."""
    return [{"s": np.ascontiguousarray(fS[B_LOC * i:B_LOC * (i + 1)]),
             "t": np.ascontiguousarray(fT[B_LOC * i:B_LOC * (i + 1)])}
            for i in range(N_CORES)]


def finish(pooled_list):
    """Host epilogue: reassemble pooled features, gram + normalize + loss."""
    B = B_LOC * N_CORES
    fS = np.full((B, C, 4), -np.inf)
    fT = np.full((B, C, 4), -np.inf)
    metas = _xfer_meta()
    for i, arr in enumerate(pooled_list):
        a = np.asarray(arr)  # [P, N_COLS]; cols 2k,2k+1 = quadrant pair
        f = (fS, fT)
        for k, (bi, _r_off, _rows) in enumerate(metas):
            xi, bl, cb, band = _band_addr(bi)
            tgt = f[xi][i * B_LOC + bl, cb * P:(cb + 1) * P,
                        band * 2:band * 2 + 2]
            np.maximum(tgt, a[:, 2 * k:2 * k + 2], out=tgt)

    def sim(f):
        G = np.einsum('bcm,bcn->bmn', f, f)
        d = np.sqrt(np.einsum('bmm->bm', G)) + 1e-8
        return G / (d[:, :, None] * d[:, None, :])

    loss = ((sim(fT) - sim(fS)) ** 2).sum() / (4 * 4) / B
    return np.float32(loss)


def run_device(fS, fT, **spmd_kwargs):
    """Run the compiled program on the 8 cores; returns (pooled_list, results)."""
    res = run_bass_kernel_spmd(get_nc(), make_in_maps(fS, fT),
                               core_ids=list(range(N_CORES)), **spmd_kwargs)
    pooled_list = [res.results[i]["pooled"] for i in range(N_CORES)]
    return pooled_list, res


def kernel(preds_S, preds_T, feat_ind):
    fi = int(np.asarray(feat_ind))
    fS = np.ascontiguousarray(np.asarray(preds_S)[fi], dtype=np.float32)
    fT = np.ascontiguousarray(np.asarray(preds_T)[fi], dtype=np.float32)
    try:
        pooled_list, _ = run_device(fS, fT)
    except Exception:
        # one retry: a cold device occasionally reports a transient
        # NRT execution error on the very first NEFF launch
        pooled_list, _ = run_device(fS, fT)
    return finish(pooled_list)


# revision 11
# speedup vs baseline: 14.2386x; 1.0543x over previous
"""Trainium2 Bass kernel for nn_CriterionPairWiseforWholeFeatAfterPool.

Computation (reference): select feat_ind slice -> MaxPool2d with kernel
(H/2, W/2) producing a 2x2 pooled map per (sample, channel) -> L2-normalize
over channels -> per-sample 4x4 gram over the pooled spatial positions ->
scalar MSE-style loss between teacher/student grams.

Strategy (data-parallel, per the sharding hint): shard the batch axis B=16
across 8 NeuronCores (2 samples/core).  Each core streams its two feature
shards (2 samples x 256 ch x 128 x 128 f32 = 64 MiB) HBM->SBUF with
channels on partitions and reduces every 64x64 max-pool window on the
vector engine (free-axis reduce_max over a strided quadrant view).  Each
core emits its pooled (partial-max) features; the tiny epilogue (fold
partials, per-sample 4x4 gram, normalization, final sum == the all-reduce
of per-core partials) runs on host in a few microseconds of numpy.

Chunking: every pooling band streams as 4 slim 16-row chunks (1 MiB DMA
ops, 8 KiB contiguous per partition -> 128 descriptors, perfectly flat
across the 16 SDMA engines).  Slim chunks keep the vector engine's reduce
granularity small, so it tracks the stream with <=1-chunk (~2.4 us) lag:
bytes start flowing ~2 us after block entry and the final reduce lands
~2.5 us after the last byte.  NBUF=18 slots (18 MiB of stream in flight)
absorb the DVE's mid-stream jitter so the DMA queue never stalls on slot
reuse (the old 12-slot version lost ~9 us to those stalls).  The block
skips the gpsimd dge-drain at exit (gpsimd unused).

Known hardware asymmetry (measured, not fixable in-kernel): on even
physical NeuronCores one edge SDMA engine (idx 0 or 15) runs ~20% slower
when the pair-sharing core is also streaming; HWDGE splits every op's
descriptors positionally (engine = position % 16 from 0), so no op shape
can give the edge engines a smaller byte share without starving the
middle engines first — those cores are bound at ~195-215 us while the
clean cores finish in ~170 us.
"""

import contextlib

import numpy as np

import concourse.bacc as bacc
import concourse.mybir as mybir
from concourse.bass_utils import run_bass_kernel_spmd

N_CORES = 8
P = 128           # SBUF partitions
B_LOC = 2         # samples per core (16 / 8)
C = 256           # channels
H = 128
W = 128
BAND = 64         # pooling-window rows
SPLIT = 4         # chunks per band (16 rows, 1 MiB each)
ROWS = BAND // SPLIT
FREE = ROWS * W   # f32 elements per partition per chunk (8 KiB)
NBUF = 18         # SBUF slots (18 x 8 KiB/partition = 144 KiB)

N_BANDS = B_LOC * 2 * (C // P) * (H // BAND)   # 16 bands
N_XFERS = N_BANDS * SPLIT                      # 64 slim chunks
N_COLS = N_XFERS * 2                           # pooled cols (pairs)

_NC = None


def _xfer_meta():
    """Transfer list metadata: (band_index, sub_row_offset, rows)."""
    return [(bi, k * ROWS, ROWS)
            for bi in range(N_BANDS) for k in range(SPLIT)]


def _band_addr(bi):
    """band index -> (x_idx, b, cb, band) in stream order."""
    band = bi % (H // BAND)
    r = bi // (H // BAND)
    cb = r % (C // P)
    r //= C // P
    x = r % 2
    b = r // 2
    return x, b, cb, band


def _build_nc():
    """Build + compile the per-core SPMD Bass program (same NEFF on all cores)."""
    nc = bacc.Bacc("TRN2", target_bir_lowering=False, debug=False,
                   num_devices=N_CORES)
    s = nc.dram_tensor("s", [B_LOC, C, H, W], mybir.dt.float32,
                       kind="ExternalInput").ap()
    t = nc.dram_tensor("t", [B_LOC, C, H, W], mybir.dt.float32,
                       kind="ExternalInput").ap()
    out = nc.dram_tensor("pooled", [P, N_COLS], mybir.dt.float32,
                         kind="ExternalOutput").ap()

    # transfer list: (2-D dram source AP, free elems, rows covered)
    xfers = []
    for bi, r_off, rows in _xfer_meta():
        xi, b, cb, band = _band_addr(bi)
        x = (s, t)[xi]
        r0 = band * BAND + r_off
        src = x[b, cb * P:(cb + 1) * P, r0:r0 + rows, :]
        xfers.append((src.rearrange("c h w -> c (h w)"), rows * W, rows))
    n = len(xfers)
    assert n == N_XFERS

    with contextlib.ExitStack() as ctx:
        bufs = [ctx.enter_context(
            nc.sbuf_tensor(f"buf{i}", [P, FREE], mybir.dt.float32))
            for i in range(NBUF)]
        pooled = ctx.enter_context(
            nc.sbuf_tensor("pooled_sb", [P, N_COLS], mybir.dt.float32))
        # one DMA-completion semaphore per buffer slot: at most one in-flight
        # DMA per semaphore (slot reuse is serialized through red_sem), so
        # concurrent DMAs never race on the same semaphore
        dma_sems = [ctx.enter_context(nc.semaphore(f"dma_sem{i}"))
                    for i in range(NBUF)]
        out_sem = ctx.enter_context(nc.semaphore("out_sem"))
        red_sem = ctx.enter_context(nc.semaphore("red_sem"))
        # gpsimd is unused; skip its expensive dge_drain at block exit
        block = ctx.enter_context(nc.Block(no_gpsimd_drain=True))

        @block.sync
        def _(sync):
            for i, (src, free, _h) in enumerate(xfers):
                if i >= NBUF:
                    # slot reuse: wait until the reduce of tile i-NBUF is done
                    sync.wait_ge(red_sem, i - NBUF + 1)
                sync.dma_start(
                    bufs[i % NBUF][:, :free], src).then_inc(
                        dma_sems[i % NBUF], 16)
            sync.wait_ge(red_sem, n)
            sync.dma_start(out, pooled[:, :]).then_inc(out_sem, 16)
            sync.wait_ge(out_sem, 16)

        @block.vector
        def _(vector):
            for i, (_src, free, h) in enumerate(xfers):
                vector.wait_ge(dma_sems[i % NBUF], 16 * (i // NBUF + 1))
                # free dim is (h, w) row-major; expose the two 64-wide halves
                # as an outer axis, reduce the h x 64 window per half
                view = bufs[i % NBUF][:, :free].rearrange(
                    "c (h j w) -> c j h w", h=h, j=2, w=64)
                vector.tensor_reduce(
                    pooled[:, 2 * i:2 * i + 2], view,
                    axis=mybir.AxisListType.XY,
                    op=mybir.AluOpType.max).then_inc(red_sem, 1)

    nc.compile()
    return nc


def get_nc():
    global _NC
    if _NC is None:
        _NC = _build_nc()
    return _NC


def make_in_maps(fS, fT):
    """Per-core input dicts: batch-sharded contiguous slices."""
    return [{"s": np.ascontiguousarray(fS[B_LOC * i:B_LOC * (i + 1)]),
             "t": np.ascontiguousarray(fT[B_LOC * i:B_LOC * (i + 1)])}
            for i in range(N_CORES)]


def finish(pooled_list):
    """Host epilogue: reassemble pooled features, gram + normalize + loss."""
    B = B_LOC * N_CORES
    fS = np.full((B, C, 4), -np.inf)
    fT = np.full((B, C, 4), -np.inf)
    metas = _xfer_meta()
    for i, arr in enumerate(pooled_list):
        a = np.asarray(arr)  # [P, N_COLS]; cols 2k,2k+1 = quadrant pair
        f = (fS, fT)
        for k, (bi, _r_off, _rows) in enumerate(metas):
            xi, bl, cb, band = _band_addr(bi)
            tgt = f[xi][i * B_LOC + bl, cb * P:(cb + 1) * P,
                        band * 2:band * 2 + 2]
            np.maximum(tgt, a[:, 2 * k:2 * k + 2], out=tgt)

    def sim(f):
        G = np.einsum('bcm,bcn->bmn', f, f)
        d = np.sqrt(np.einsum('bmm->bm', G)) + 1e-8
        return G / (d[:, :, None] * d[:, None, :])

    loss = ((sim(fT) - sim(fS)) ** 2).sum() / (4 * 4) / B
    return np.float32(loss)


def run_device(fS, fT, **spmd_kwargs):
    """Run the compiled program on the 8 cores; returns (pooled_list, results)."""
    res = run_bass_kernel_spmd(get_nc(), make_in_maps(fS, fT),
                               core_ids=list(range(N_CORES)), **spmd_kwargs)
    pooled_list = [res.results[i]["pooled"] for i in range(N_CORES)]
    return pooled_list, res


def kernel(preds_S, preds_T, feat_ind):
    fi = int(np.asarray(feat_ind))
    fS = np.ascontiguousarray(np.asarray(preds_S)[fi], dtype=np.float32)
    fT = np.ascontiguousarray(np.asarray(preds_T)[fi], dtype=np.float32)
    try:
        pooled_list, _ = run_device(fS, fT)
    except Exception:
        # one retry: a cold device occasionally reports a transient
        # NRT execution error on the very first NEFF launch
        pooled_list, _ = run_device(fS, fT)
    return finish(pooled_list)
